# revision 15
# baseline (speedup 1.0000x reference)
"""Trainium2 Bass kernel: atrous (dilated) multi-head attention block.

Computation (per reference):
  x [2, 4096, 1024] --atrous regroup (dil=4)--> xr [8, 1024, 1024]
  q/k/v = xr @ W{q,k,v}.T + b;  16 heads, dh=64
  probs = softmax(q k^T / 8);  ctx = probs v
  atted = ctx @ Wf.T + bf;  final = LN(atted + x)
  returns (final, atted)

Sharding: B*dil == 8 == n_cores, so each NeuronCore takes one atrous group
[1024, 1024] — pure data parallel, zero collectives. The host performs the
strided regroup/scatter (that IS the shard selection) and pre-transposes /
pre-scales operands so the device kernel needs no on-chip transposes:

  per-core device layout (all matmuls contract over the partition dim):
    xT  [D, L] bf16   : lhsT for V-proj, rhs for Q/K-proj
    qT/kT [D, L] bf16 : head-transposed; scoresT_h = kT_h^T @ qT_h (K=dh=64)
    expT = exp(scoresT) (Wq pre-scaled by 1/8 on host)
    v_aug [L, H, 65] bf16 : v with an appended ones-column per head, so the
        ctx matmul (lhsT=v_aug, rhs=expT) also yields softmax denominators
        in psum row 64.  ctxT normalized via reciprocal + DMA broadcast.
    atted = ctxT^T @ WfT (K=D) in natural [L, D] layout; +bias, +x, LN.
"""

import os
import sys
from contextlib import ExitStack

for _p in ("/opt/trn_rl_repo",):
    if os.path.isdir(_p) and _p not in sys.path:
        sys.path.insert(0, _p)

import numpy as np
import ml_dtypes

import concourse.bass as bass
import concourse.mybir as mybir
from concourse.tile import TileContext
from concourse.bass_utils import run_bass_kernel_spmd

B, S, D = 2, 4096, 1024
DIL = 4
NCORES = 8
L = S // DIL  # 1024 rows per core
H, DH = 16, 64
P = 128
KC = D // P  # 8 contraction chunks
MT = D // P  # 8 output chunks
NT = 512  # matmul free-dim tile
EPS = 1e-5
SCALE = 1.0 / 8.0  # 1/sqrt(dh)

F32 = mybir.dt.float32
BF16 = mybir.dt.bfloat16
AL = mybir.AluOpType
AF = mybir.ActivationFunctionType
BF16_NP = ml_dtypes.bfloat16


def _split_excess_waits(nc: bass.Bass, max_waits: int = 1) -> None:
    """This neuronxcc's walrus rejects instructions carrying more than
    `max_waits` semaphore waits ("Too many sync wait commands").  Tile's
    kernel-tail drain (and occasionally a compute op) can exceed that.
    Move the excess waits onto same-engine no-ops inserted just before the
    instruction — the engine executes in order, so the happens-before
    relation is preserved exactly."""
    n = 0
    for fn in nc.m.functions:
        for blk in fn.blocks:
            insts = list(blk.instructions)
            out = []
            changed = False
            for inst in insts:
                si = inst.sync_info
                waits = list(si.on_wait) if (si is not None and si.on_wait) else []
                if len(waits) > max_waits:
                    changed = True
                    excess, keep = waits[:-max_waits], waits[-max_waits:]
                    for i in range(0, len(excess), max_waits):
                        nop = mybir.InstNoOp(name=f"waitsplit-{n}", ins=[], outs=[])
                        n += 1
                        nop.engine = inst.engine
                        nop.sync_info = mybir.SyncInfo(
                            on_wait=excess[i : i + max_waits], on_update=[]
                        )
                        nc.register_instruction(nop)
                        out.append(nop)
                    si.on_wait = keep
                out.append(inst)
            if changed:
                blk.instructions = out


def build_graph() -> bass.Bass:
    nc = bass.Bass()
    xT_e = nc.declare_dram_parameter("xT", [D, L], BF16, isOutput=False)
    xn_e = nc.declare_dram_parameter("xn", [L, D], F32, isOutput=False)
    wq_e = nc.declare_dram_parameter("wqT", [D, D], BF16, isOutput=False)
    wk_e = nc.declare_dram_parameter("wkT", [D, D], BF16, isOutput=False)
    wv_e = nc.declare_dram_parameter("wvT", [D, D], BF16, isOutput=False)
    wf_e = nc.declare_dram_parameter("wfT", [D, D], BF16, isOutput=False)
    bqc_e = nc.declare_dram_parameter("bqc", [P, MT], F32, isOutput=False)
    bkc_e = nc.declare_dram_parameter("bkc", [P, MT], F32, isOutput=False)
    bv_e = nc.declare_dram_parameter("bv", [D], F32, isOutput=False)
    bf_e = nc.declare_dram_parameter("bf", [D], F32, isOutput=False)
    gam_e = nc.declare_dram_parameter("gam", [D], F32, isOutput=False)
    bet_e = nc.declare_dram_parameter("bet", [D], F32, isOutput=False)
    out_e = nc.declare_dram_parameter("out", [2, L, D], F32, isOutput=True)

    with TileContext(nc) as tc, ExitStack() as ctx:
        const = ctx.enter_context(tc.tile_pool(name="const", bufs=1))
        persist = ctx.enter_context(tc.tile_pool(name="persist", bufs=1))
        wpool = ctx.enter_context(tc.tile_pool(name="wpool", bufs=2))
        mmps = ctx.enter_context(tc.tile_pool(name="mmps", bufs=2, space="PSUM"))
        spool = ctx.enter_context(tc.tile_pool(name="spool", bufs=2, space="PSUM"))
        epool = ctx.enter_context(tc.tile_pool(name="epool", bufs=2))
        npool = ctx.enter_context(tc.tile_pool(name="npool", bufs=2))
        xpool = ctx.enter_context(tc.tile_pool(name="xpool", bufs=2))
        dpool = ctx.enter_context(tc.tile_pool(name="dpool", bufs=2, space="DRAM"))
        opool = ctx.enter_context(tc.tile_pool(name="opool", bufs=2))
        stat = ctx.enter_context(tc.tile_pool(name="stat", bufs=4))

        # ---- constants / whole-kernel inputs
        xT_sb = persist.tile([P, KC, L], BF16, tag="xT")
        nc.sync.dma_start(out=xT_sb[:], in_=xT_e.rearrange("(kc p) l -> p kc l", p=P))
        bqc_sb = const.tile([P, MT], F32, tag="bqc")
        nc.sync.dma_start(out=bqc_sb[:], in_=bqc_e[:])
        bkc_sb = const.tile([P, MT], F32, tag="bkc")
        nc.sync.dma_start(out=bkc_sb[:], in_=bkc_e[:])
        bvb = const.tile([P, D], F32, tag="cb1")
        nc.sync.dma_start(out=bvb[:], in_=bv_e[None, :].to_broadcast((P, D)))
        bfb = const.tile([P, D], F32, tag="bfb")
        nc.sync.dma_start(out=bfb[:], in_=bf_e[None, :].to_broadcast((P, D)))
        gmb = const.tile([P, D], F32, tag="cb1")
        nc.sync.dma_start(out=gmb[:], in_=gam_e[None, :].to_broadcast((P, D)))
        btb = const.tile([P, D], F32, tag="btb")
        nc.sync.dma_start(out=btb[:], in_=bet_e[None, :].to_broadcast((P, D)))
        epsb = const.tile([P, 1], F32, tag="epsb")
        nc.vector.memset(epsb[:], EPS)

        # per-chunk persistent arrays (separate tiles => fine-grained deps)
        qT = [persist.tile([P, L], BF16, tag=f"qT{m}", name=f"qT{m}") for m in range(MT)]
        kT = [persist.tile([P, L], BF16, tag=f"kT{m}", name=f"kT{m}") for m in range(MT)]
        vA = [persist.tile([P, H, DH + 1], BF16, tag=f"vA{m}", name=f"vA{m}") for m in range(KC)]
        cT = [persist.tile([P, L], BF16, tag=f"cT{m}", name=f"cT{m}") for m in range(KC)]
        for m in range(KC):
            nc.vector.memset(vA[m][:, :, DH : DH + 1], 1.0)

        # ---- weight loads (wv/wq early; wk reuses wv's slot, wf reuses wq's)
        def load_w(e):
            w = wpool.tile([P, KC, D], BF16, tag="w")
            nc.sync.dma_start(out=w[:], in_=e.rearrange("(kc p) n -> p kc n", p=P))
            return w

        wv_sb = load_w(wv_e)
        wq_sb = load_w(wq_e)

        # ---- V projection first (unblocks per-head ctx matmuls early)
        # v natural layout: psum[l_chunk, d_out] = xT^T @ WvT
        for m in range(MT):
            for t in range(2):
                ps = mmps.tile([P, NT], F32, tag="mm")
                for kc in range(KC):
                    nc.tensor.matmul(
                        ps[:],
                        xT_sb[:, kc, m * P : (m + 1) * P],
                        wv_sb[:, kc, t * NT : (t + 1) * NT],
                        start=(kc == 0),
                        stop=(kc == KC - 1),
                    )
                dest = vA[m][:, t * 8 : (t + 1) * 8, 0:DH]
                nc.vector.tensor_tensor(
                    dest,
                    ps[:].rearrange("p (h e) -> p h e", e=DH),
                    bvb[:, t * NT : (t + 1) * NT].rearrange("p (h e) -> p h e", e=DH),
                    AL.add,
                )

        wk_sb = load_w(wk_e)

        # ---- Q/K projections, head-transposed: psum[d_out_chunk, l] = WT^T @ xT
        for m in range(MT):
            for w_sb, bias_sb, dst in ((wq_sb, bqc_sb, qT), (wk_sb, bkc_sb, kT)):
                for t in range(2):
                    ps = mmps.tile([P, NT], F32, tag="mm")
                    for kc in range(KC):
                        nc.tensor.matmul(
                            ps[:],
                            w_sb[:, kc, m * P : (m + 1) * P],
                            xT_sb[:, kc, t * NT : (t + 1) * NT],
                            start=(kc == 0),
                            stop=(kc == KC - 1),
                        )
                    nc.vector.tensor_scalar_add(
                        dst[m][:, t * NT : (t + 1) * NT], ps[:], bias_sb[:, m : m + 1]
                    )

        # ---- attention, one head at a time
        for h in range(H):
            hp = (h % 2) * DH  # partition base of this head inside its chunk
            hc = h // 2  # which 128-row chunk holds this head
            eT = epool.tile([P, KC, L], BF16, tag="eT")
            for jc in range(KC):
                ps = spool.tile([P, L], F32, tag="sc")
                for t in range(2):
                    nc.tensor.matmul(
                        ps[:, t * NT : (t + 1) * NT],
                        kT[hc][hp : hp + DH, jc * P : (jc + 1) * P],
                        qT[hc][hp : hp + DH, t * NT : (t + 1) * NT],
                        start=True,
                        stop=True,
                    )
                nc.scalar.activation(eT[:, jc, :], ps[:], AF.Exp)
            for t in range(2):
                pc = mmps.tile([P, NT], F32, tag="mm")
                for jc in range(KC):
                    nc.tensor.matmul(
                        pc[0 : DH + 1, :],
                        vA[jc][:, h, :],
                        eT[:, jc, t * NT : (t + 1) * NT],
                        start=(jc == 0),
                        stop=(jc == KC - 1),
                    )
                # softmax denominators sit in psum row DH; normalize rows 0..DH-1
                rr = npool.tile([1, NT], F32, tag="rr")
                nc.vector.reciprocal(rr[:], pc[DH : DH + 1, :])
                rb = npool.tile([DH, NT], F32, tag="rb")
                rd = dpool.tile([1, NT], F32, tag="rd")
                nc.sync.dma_start(out=rd[:], in_=rr[:])
                nc.sync.dma_start(out=rb[:], in_=rd[:].to_broadcast((DH, NT)))
                nc.vector.tensor_tensor(
                    cT[hc][hp : hp + DH, t * NT : (t + 1) * NT],
                    pc[0:DH, :],
                    rb[:],
                    AL.mult,
                )

        # ---- output projection + residual + layernorm
        wf_sb = load_w(wf_e)
        for m in range(MT):
            xn_t = xpool.tile([P, D], F32, tag="xn")
            nc.sync.dma_start(out=xn_t[:], in_=xn_e[m * P : (m + 1) * P, :])
            att = opool.tile([P, D], F32, tag="att")
            for t in range(2):
                ps = mmps.tile([P, NT], F32, tag="mm")
                for kc in range(KC):
                    nc.tensor.matmul(
                        ps[:],
                        cT[kc][:, m * P : (m + 1) * P],
                        wf_sb[:, kc, t * NT : (t + 1) * NT],
                        start=(kc == 0),
                        stop=(kc == KC - 1),
                    )
                nc.vector.tensor_tensor(
                    att[:, t * NT : (t + 1) * NT],
                    ps[:],
                    bfb[:, t * NT : (t + 1) * NT],
                    AL.add,
                )
            nc.sync.dma_start(out=out_e[1, m * P : (m + 1) * P, :], in_=att[:])
            # LN epilogue, in-place in `att` (waits on the atted DMA-out above)
            ssum = stat.tile([P, 1], F32, tag="ss")
            nc.vector.scalar_tensor_tensor(
                att[:], att[:], 1.0, xn_t[:], AL.mult, AL.add, accum_out=ssum[:]
            )
            sq = opool.tile([P, D], F32, tag="sq")
            sqs = stat.tile([P, 1], F32, tag="sqs")
            nc.vector.tensor_tensor(sq[:], att[:], att[:], AL.mult)
            nc.vector.tensor_reduce(sqs[:], sq[:], mybir.AxisListType.X, AL.add)
            mu = stat.tile([P, 1], F32, tag="mu")
            nc.vector.tensor_scalar_mul(mu[:], ssum[:], 1.0 / D)
            ex2 = stat.tile([P, 1], F32, tag="ex2")
            nc.vector.tensor_scalar_mul(ex2[:], sqs[:], 1.0 / D)
            msq = stat.tile([P, 1], F32, tag="msq")
            nc.vector.tensor_tensor(msq[:], mu[:], mu[:], AL.mult)
            var = stat.tile([P, 1], F32, tag="var")
            nc.vector.tensor_sub(var[:], ex2[:], msq[:])
            sd = stat.tile([P, 1], F32, tag="sd")
            nc.scalar.activation(sd[:], var[:], AF.Sqrt, bias=epsb[:])
            inv = stat.tile([P, 1], F32, tag="inv")
            nc.vector.reciprocal(inv[:], sd[:])
            nc.vector.tensor_scalar(att[:], att[:], mu[:], inv[:], AL.subtract, AL.mult)
            nc.vector.scalar_tensor_tensor(
                att[:], att[:], 1.0, gmb[:], AL.mult, AL.mult
            )
            nc.vector.tensor_tensor(att[:], att[:], btb[:], AL.add)
            nc.sync.dma_start(out=out_e[0, m * P : (m + 1) * P, :], in_=att[:])

    _split_excess_waits(nc)
    return nc


def prepare_in_maps(inputs):
    x = np.asarray(inputs["x"], np.float32)
    xr = x.reshape(B, L, DIL, D).transpose(0, 2, 1, 3).reshape(NCORES, L, D)
    shared = {
        "wqT": (np.asarray(inputs["Wq"], np.float32).T * SCALE).astype(BF16_NP),
        "wkT": np.asarray(inputs["Wk"], np.float32).T.astype(BF16_NP),
        "wvT": np.asarray(inputs["Wv"], np.float32).T.astype(BF16_NP),
        "wfT": np.asarray(inputs["Wf"], np.float32).T.astype(BF16_NP),
        "bqc": np.ascontiguousarray(
            (np.asarray(inputs["bq"], np.float32) * SCALE).reshape(MT, P).T
        ),
        "bkc": np.ascontiguousarray(
            np.asarray(inputs["bk"], np.float32).reshape(MT, P).T
        ),
        "bv": np.ascontiguousarray(inputs["bv"], dtype=np.float32),
        "bf": np.ascontiguousarray(inputs["bf"], dtype=np.float32),
        "gam": np.ascontiguousarray(inputs["gamma"], dtype=np.float32),
        "bet": np.ascontiguousarray(inputs["beta"], dtype=np.float32),
    }
    maps = []
    for c in range(NCORES):
        xs = np.ascontiguousarray(xr[c])
        m = dict(shared)
        m["xT"] = xs.T.astype(BF16_NP)
        m["xn"] = xs
        maps.append(m)
    return maps


def gather_outputs(results):
    outs = np.stack([np.asarray(results[c]["out"]) for c in range(NCORES)])
    final = outs[:, 0].reshape(B, DIL, L, D).transpose(0, 2, 1, 3).reshape(B, S, D)
    atted = outs[:, 1].reshape(B, DIL, L, D).transpose(0, 2, 1, 3).reshape(B, S, D)
    return np.ascontiguousarray(final), np.ascontiguousarray(atted)


_GRAPH = None


def get_graph():
    global _GRAPH
    if _GRAPH is None:
        _GRAPH = build_graph()
    return _GRAPH


def run(inputs, trace=False, **kw):
    nc = get_graph()
    maps = prepare_in_maps(inputs)
    res = run_bass_kernel_spmd(nc, maps, core_ids=list(range(NCORES)), trace=trace, **kw)
    return gather_outputs(res.results), res


def kernel(**inputs):
    (final, atted), _ = run(inputs, trace=False)
    return final, atted


# revision 21
# speedup vs baseline: 1.1792x; 1.1792x over previous
"""Trainium2 Bass kernel: atrous (dilated) multi-head attention block.

Computation (per reference):
  x [2, 4096, 1024] --atrous regroup (dil=4)--> xr [8, 1024, 1024]
  q/k/v = xr @ W{q,k,v}.T + b;  16 heads, dh=64
  probs = softmax(q k^T / 8);  ctx = probs v
  atted = ctx @ Wf.T + bf;  final = LN(atted + x)
  returns (final, atted)

Sharding: B*dil == 8 == n_cores, so each NeuronCore takes one atrous group
[1024, 1024] — pure data parallel, zero collectives. The host performs the
strided regroup/scatter (that IS the shard selection) and pre-transposes /
pre-scales operands so the device kernel needs no on-chip transposes:

  per-core device layout (all matmuls contract over the partition dim):
    xT  [D, L] bf16   : lhsT for V-proj, rhs for Q/K-proj
    qT/kT [D, L] bf16 : head-transposed; scoresT_h = kT_h^T @ qT_h (K=dh=64)
    expT = exp(scoresT) (Wq pre-scaled by 1/8 on host)
    v_aug [L, H, 65] bf16 : v with an appended ones-column per head, so the
        ctx matmul (lhsT=v_aug, rhs=expT) also yields softmax denominators
        in psum row 64.  ctxT normalized via reciprocal + DMA broadcast.
    atted = ctxT^T @ WfT (K=D) in natural [L, D] layout; +bias, +x, LN.
"""

import os
import sys
from contextlib import ExitStack

for _p in ("/opt/trn_rl_repo",):
    if os.path.isdir(_p) and _p not in sys.path:
        sys.path.insert(0, _p)

import numpy as np
import ml_dtypes

import concourse.bass as bass
import concourse.mybir as mybir
from concourse.tile import TileContext
from concourse.bass_utils import run_bass_kernel_spmd

B, S, D = 2, 4096, 1024
DIL = 4
NCORES = 8
L = S // DIL  # 1024 rows per core
H, DH = 16, 64
P = 128
KC = D // P  # 8 contraction chunks
MT = D // P  # 8 output chunks
NT = 512  # matmul free-dim tile
EPS = 1e-5
SCALE = 1.0 / 8.0  # 1/sqrt(dh)

F32 = mybir.dt.float32
BF16 = mybir.dt.bfloat16
AL = mybir.AluOpType
AF = mybir.ActivationFunctionType
BF16_NP = ml_dtypes.bfloat16


def _split_excess_waits(nc: bass.Bass, max_waits: int = 1) -> None:
    """This neuronxcc's walrus rejects instructions carrying more than
    `max_waits` semaphore waits ("Too many sync wait commands").  Tile's
    kernel-tail drain (and occasionally a compute op) can exceed that.
    Move the excess waits onto same-engine no-ops inserted just before the
    instruction — the engine executes in order, so the happens-before
    relation is preserved exactly."""
    n = 0
    for fn in nc.m.functions:
        for blk in fn.blocks:
            insts = list(blk.instructions)
            out = []
            changed = False
            for inst in insts:
                si = inst.sync_info
                waits = list(si.on_wait) if (si is not None and si.on_wait) else []
                if len(waits) > max_waits:
                    changed = True
                    excess, keep = waits[:-max_waits], waits[-max_waits:]
                    for i in range(0, len(excess), max_waits):
                        nop = mybir.InstNoOp(name=f"waitsplit-{n}", ins=[], outs=[])
                        n += 1
                        nop.engine = inst.engine
                        nop.sync_info = mybir.SyncInfo(
                            on_wait=excess[i : i + max_waits], on_update=[]
                        )
                        nc.register_instruction(nop)
                        out.append(nop)
                    si.on_wait = keep
                out.append(inst)
            if changed:
                blk.instructions = out


def build_graph() -> bass.Bass:
    nc = bass.Bass()
    xT_e = nc.declare_dram_parameter("xT", [D, L], BF16, isOutput=False)
    xn_e = nc.declare_dram_parameter("xn", [L, D], F32, isOutput=False)
    wq_e = nc.declare_dram_parameter("wqT", [D, D], BF16, isOutput=False)
    wk_e = nc.declare_dram_parameter("wkT", [D, D], BF16, isOutput=False)
    wv_e = nc.declare_dram_parameter("wvT", [D, D], BF16, isOutput=False)
    wf_e = nc.declare_dram_parameter("wfT", [D, D], BF16, isOutput=False)
    bqc_e = nc.declare_dram_parameter("bqc", [P, MT], F32, isOutput=False)
    bkc_e = nc.declare_dram_parameter("bkc", [P, MT], F32, isOutput=False)
    bv_e = nc.declare_dram_parameter("bv", [D], F32, isOutput=False)
    bf_e = nc.declare_dram_parameter("bf", [D], F32, isOutput=False)
    gam_e = nc.declare_dram_parameter("gam", [D], F32, isOutput=False)
    bet_e = nc.declare_dram_parameter("bet", [D], F32, isOutput=False)
    out_e = nc.declare_dram_parameter("out", [2, L, D], F32, isOutput=True)

    with TileContext(nc) as tc, ExitStack() as ctx:
        const = ctx.enter_context(tc.tile_pool(name="const", bufs=1))
        persist = ctx.enter_context(tc.tile_pool(name="persist", bufs=1))
        wpool = ctx.enter_context(tc.tile_pool(name="wpool", bufs=2))
        mmps = ctx.enter_context(tc.tile_pool(name="mmps", bufs=2, space="PSUM"))
        spool = ctx.enter_context(tc.tile_pool(name="spool", bufs=3, space="PSUM"))
        epool = ctx.enter_context(tc.tile_pool(name="epool", bufs=2))
        npool = ctx.enter_context(tc.tile_pool(name="npool", bufs=2))
        xpool = ctx.enter_context(tc.tile_pool(name="xpool", bufs=2))
        dpool = ctx.enter_context(tc.tile_pool(name="dpool", bufs=2, space="DRAM"))
        opool = ctx.enter_context(tc.tile_pool(name="opool", bufs=2))
        stat = ctx.enter_context(tc.tile_pool(name="stat", bufs=4))

        # ---- constants / whole-kernel inputs
        xT_sb = persist.tile([P, KC, L], BF16, tag="xT")
        xT_src = xT_e.rearrange("(kc p) l -> p kc l", p=P)
        for kc in range(KC):
            nc.sync.dma_start(
                out=xT_sb[:, kc : kc + 1, :], in_=xT_src[:, kc : kc + 1, :]
            )
        bqc_sb = const.tile([P, MT], F32, tag="bqc")
        nc.sync.dma_start(out=bqc_sb[:], in_=bqc_e[:])
        bkc_sb = const.tile([P, MT], F32, tag="bkc")
        nc.sync.dma_start(out=bkc_sb[:], in_=bkc_e[:])
        bvb = const.tile([P, D], F32, tag="cb1")
        nc.sync.dma_start(out=bvb[:], in_=bv_e[None, :].to_broadcast((P, D)))
        bfb = const.tile([P, D], F32, tag="bfb")
        nc.sync.dma_start(out=bfb[:], in_=bf_e[None, :].to_broadcast((P, D)))
        gmb = const.tile([P, D], F32, tag="cb1")
        nc.sync.dma_start(out=gmb[:], in_=gam_e[None, :].to_broadcast((P, D)))
        btb = const.tile([P, D], F32, tag="btb")
        nc.sync.dma_start(out=btb[:], in_=bet_e[None, :].to_broadcast((P, D)))
        epsb = const.tile([P, 1], F32, tag="epsb")
        nc.vector.memset(epsb[:], EPS)

        # per-chunk persistent arrays (separate tiles => fine-grained deps)
        qT = [persist.tile([P, L], BF16, tag=f"qT{m}", name=f"qT{m}") for m in range(MT)]
        kT = [persist.tile([P, L], BF16, tag=f"kT{m}", name=f"kT{m}") for m in range(MT)]
        vA = [persist.tile([P, H, DH + 1], BF16, tag=f"vA{m}", name=f"vA{m}") for m in range(KC)]
        cT = [persist.tile([P, L], BF16, tag=f"cT{m}", name=f"cT{m}") for m in range(KC)]
        for m in range(KC):
            nc.vector.memset(vA[m][:, :, DH : DH + 1], 1.0)

        # ---- weight loads (wv/wq early; wk reuses wv's slot, wf reuses wq's)
        # split per-kc so the chunks land on parallel DMA queues
        def load_w(e):
            w = wpool.tile([P, KC, D], BF16, tag="w")
            src = e.rearrange("(kc p) n -> p kc n", p=P)
            for kc in range(KC):
                nc.sync.dma_start(
                    out=w[:, kc : kc + 1, :], in_=src[:, kc : kc + 1, :]
                )
            return w

        wv_sb = load_w(wv_e)
        wq_sb = load_w(wq_e)

        # ---- V projection first (unblocks per-head ctx matmuls early)
        # v natural layout: psum[l_chunk, d_out] = xT^T @ WvT
        for m in range(MT):
            for t in range(2):
                ps = mmps.tile([P, NT], F32, tag="mm")
                for kc in range(KC):
                    nc.tensor.matmul(
                        ps[:],
                        xT_sb[:, kc, m * P : (m + 1) * P],
                        wv_sb[:, kc, t * NT : (t + 1) * NT],
                        start=(kc == 0),
                        stop=(kc == KC - 1),
                    )
                dest = vA[m][:, t * 8 : (t + 1) * 8, 0:DH]
                nc.vector.tensor_tensor(
                    dest,
                    ps[:].rearrange("p (h e) -> p h e", e=DH),
                    bvb[:, t * NT : (t + 1) * NT].rearrange("p (h e) -> p h e", e=DH),
                    AL.add,
                )

        wk_sb = load_w(wk_e)

        # ---- Q/K projections, head-transposed: psum[d_out_chunk, l] = WT^T @ xT
        for m in range(MT):
            for w_sb, bias_sb, dst in ((wq_sb, bqc_sb, qT), (wk_sb, bkc_sb, kT)):
                for t in range(2):
                    ps = mmps.tile([P, NT], F32, tag="mm")
                    for kc in range(KC):
                        nc.tensor.matmul(
                            ps[:],
                            w_sb[:, kc, m * P : (m + 1) * P],
                            xT_sb[:, kc, t * NT : (t + 1) * NT],
                            start=(kc == 0),
                            stop=(kc == KC - 1),
                        )
                    # bias-add + bf16 cast on ScalarE (idle during projections)
                    nc.scalar.activation(
                        dst[m][:, t * NT : (t + 1) * NT],
                        ps[:],
                        AF.Identity,
                        bias=bias_sb[:, m : m + 1],
                    )

        # ---- attention, one head at a time
        for h in range(H):
            hp = (h % 2) * DH  # partition base of this head inside its chunk
            hc = h // 2  # which 128-row chunk holds this head
            eT = epool.tile([P, KC, L], BF16, tag="eT")
            for jc in range(KC):
                ps = spool.tile([P, L], F32, tag="sc")
                for t in range(2):
                    nc.tensor.matmul(
                        ps[:, t * NT : (t + 1) * NT],
                        kT[hc][hp : hp + DH, jc * P : (jc + 1) * P],
                        qT[hc][hp : hp + DH, t * NT : (t + 1) * NT],
                        start=True,
                        stop=True,
                    )
                nc.scalar.activation(eT[:, jc, :], ps[:], AF.Exp)
            for t in range(2):
                pc = mmps.tile([P, NT], F32, tag="mm")
                for jc in range(KC):
                    nc.tensor.matmul(
                        pc[0 : DH + 1, :],
                        vA[jc][:, h, :],
                        eT[:, jc, t * NT : (t + 1) * NT],
                        start=(jc == 0),
                        stop=(jc == KC - 1),
                    )
                # softmax denominators sit in psum row DH; copy ctx + sums out
                # immediately so the psum slot frees, then reciprocal on a
                # [128, 4] reshape (all partitions active) via a DRAM bounce.
                cn = npool.tile([DH, NT], F32, tag="cn")
                nc.vector.tensor_copy(out=cn[:], in_=pc[0:DH, :])
                row = npool.tile([1, NT], F32, tag="row")
                nc.vector.tensor_copy(out=row[:], in_=pc[DH : DH + 1, :])
                rdA = dpool.tile([1, NT], F32, tag="rdA")
                nc.sync.dma_start(out=rdA[:], in_=row[:])
                st = npool.tile([P, NT // P], F32, tag="st")
                nc.sync.dma_start(
                    out=st[:], in_=rdA[0, :].rearrange("(p f) -> p f", p=P)
                )
                stR = npool.tile([P, NT // P], F32, tag="stR")
                nc.vector.reciprocal(stR[:], st[:])
                rdB = dpool.tile([1, NT], F32, tag="rdB")
                nc.sync.dma_start(
                    out=rdB[0, :].rearrange("(p f) -> p f", p=P), in_=stR[:]
                )
                rb = npool.tile([DH, NT], F32, tag="rb")
                nc.sync.dma_start(out=rb[:], in_=rdB[:].to_broadcast((DH, NT)))
                nc.vector.tensor_tensor(
                    cT[hc][hp : hp + DH, t * NT : (t + 1) * NT],
                    cn[:],
                    rb[:],
                    AL.mult,
                )

        # ---- output projection + residual + layernorm
        wf_sb = load_w(wf_e)
        for m in range(MT):
            xn_t = xpool.tile([P, D], F32, tag="xn")
            nc.sync.dma_start(out=xn_t[:], in_=xn_e[m * P : (m + 1) * P, :])
            att = opool.tile([P, D], F32, tag="att")
            for t in range(2):
                ps = mmps.tile([P, NT], F32, tag="mm")
                for kc in range(KC):
                    nc.tensor.matmul(
                        ps[:],
                        cT[kc][:, m * P : (m + 1) * P],
                        wf_sb[:, kc, t * NT : (t + 1) * NT],
                        start=(kc == 0),
                        stop=(kc == KC - 1),
                    )
                nc.vector.tensor_tensor(
                    att[:, t * NT : (t + 1) * NT],
                    ps[:],
                    bfb[:, t * NT : (t + 1) * NT],
                    AL.add,
                )
            nc.sync.dma_start(out=out_e[1, m * P : (m + 1) * P, :], in_=att[:])
            # LN epilogue, in-place in `att` (waits on the atted DMA-out above)
            ssum = stat.tile([P, 1], F32, tag="ss")
            nc.vector.scalar_tensor_tensor(
                att[:], att[:], 1.0, xn_t[:], AL.mult, AL.add, accum_out=ssum[:]
            )
            # square + row-sum on ScalarE (idle during the LN phase)
            sq = opool.tile([P, D], F32, tag="sq")
            sqs = stat.tile([P, 1], F32, tag="sqs")
            nc.scalar.activation(sq[:], att[:], AF.Square, accum_out=sqs[:])
            mu = stat.tile([P, 1], F32, tag="mu")
            nc.vector.tensor_scalar_mul(mu[:], ssum[:], 1.0 / D)
            ex2 = stat.tile([P, 1], F32, tag="ex2")
            nc.vector.tensor_scalar_mul(ex2[:], sqs[:], 1.0 / D)
            msq = stat.tile([P, 1], F32, tag="msq")
            nc.vector.tensor_tensor(msq[:], mu[:], mu[:], AL.mult)
            var = stat.tile([P, 1], F32, tag="var")
            nc.vector.tensor_sub(var[:], ex2[:], msq[:])
            sd = stat.tile([P, 1], F32, tag="sd")
            nc.scalar.activation(sd[:], var[:], AF.Sqrt, bias=epsb[:])
            inv = stat.tile([P, 1], F32, tag="inv")
            nc.vector.reciprocal(inv[:], sd[:])
            nc.vector.tensor_scalar(att[:], att[:], mu[:], inv[:], AL.subtract, AL.mult)
            nc.vector.scalar_tensor_tensor(
                att[:], att[:], 1.0, gmb[:], AL.mult, AL.mult
            )
            nc.vector.tensor_tensor(att[:], att[:], btb[:], AL.add)
            nc.sync.dma_start(out=out_e[0, m * P : (m + 1) * P, :], in_=att[:])

    _split_excess_waits(nc)
    return nc


def prepare_in_maps(inputs):
    x = np.asarray(inputs["x"], np.float32)
    xr = x.reshape(B, L, DIL, D).transpose(0, 2, 1, 3).reshape(NCORES, L, D)
    shared = {
        "wqT": (np.asarray(inputs["Wq"], np.float32).T * SCALE).astype(BF16_NP),
        "wkT": np.asarray(inputs["Wk"], np.float32).T.astype(BF16_NP),
        "wvT": np.asarray(inputs["Wv"], np.float32).T.astype(BF16_NP),
        "wfT": np.asarray(inputs["Wf"], np.float32).T.astype(BF16_NP),
        "bqc": np.ascontiguousarray(
            (np.asarray(inputs["bq"], np.float32) * SCALE).reshape(MT, P).T
        ),
        "bkc": np.ascontiguousarray(
            np.asarray(inputs["bk"], np.float32).reshape(MT, P).T
        ),
        "bv": np.ascontiguousarray(inputs["bv"], dtype=np.float32),
        "bf": np.ascontiguousarray(inputs["bf"], dtype=np.float32),
        "gam": np.ascontiguousarray(inputs["gamma"], dtype=np.float32),
        "bet": np.ascontiguousarray(inputs["beta"], dtype=np.float32),
    }
    maps = []
    for c in range(NCORES):
        xs = np.ascontiguousarray(xr[c])
        m = dict(shared)
        m["xT"] = xs.T.astype(BF16_NP)
        m["xn"] = xs
        maps.append(m)
    return maps


def gather_outputs(results):
    outs = np.stack([np.asarray(results[c]["out"]) for c in range(NCORES)])
    final = outs[:, 0].reshape(B, DIL, L, D).transpose(0, 2, 1, 3).reshape(B, S, D)
    atted = outs[:, 1].reshape(B, DIL, L, D).transpose(0, 2, 1, 3).reshape(B, S, D)
    return np.ascontiguousarray(final), np.ascontiguousarray(atted)


_GRAPH = None


def get_graph():
    global _GRAPH
    if _GRAPH is None:
        _GRAPH = build_graph()
    return _GRAPH


def run(inputs, trace=False, **kw):
    nc = get_graph()
    maps = prepare_in_maps(inputs)
    res = run_bass_kernel_spmd(nc, maps, core_ids=list(range(NCORES)), trace=trace, **kw)
    return gather_outputs(res.results), res


def kernel(**inputs):
    (final, atted), _ = run(inputs, trace=False)
    return final, atted
